# revision 5
# baseline (speedup 1.0000x reference)
"""MaxUnpooling2D scatter-add for Trainium2 (8 NeuronCores) — one-hot matmul.

Problem: updates/mask [32,112,112,64] f32/int32 -> out [32,224,224,64] f32,
out[b, y, x, c] += updates[b, h, w, c]; y,x decoded from mask. Per (b,c)
"plane": 12544 elements scatter-add into 50176 pixel bins (bin = mask>>6,
channel index = lane c).

Algorithm (NO per-element DMA): decompose bin t = lo*392 + hi (lo<128,
hi<392). For each 128-element chunk i of a plane build two one-hot fp16
matrices on the DVE (vector) engine:
    A[i, l] = (lo_i == l) * v_i        [128 x 128]   (stationary)
    M[i, h] = (hi_i == h)              [128 x 392]   (moving)
then PE matmul-accumulates PSUM[l, h] += A^T @ M over the plane's 98 chunks:
every element lands exactly at (lo_i, hi_i) with value v_i, duplicates are
summed by the contraction/PSUM accumulate (race-free by construction). The
dense plane is then copied PSUM->SBUF and written with a plain DMA.

Sharding: batch b across 8 cores x 4 sequential invocations of ONE compiled
module (one batch = 64 planes per invocation). Output is written plane-major
[c, lo, hi]; host reassembles to [b, 224*224, 64] (fixed transposes only).

Precision: one-hots are exact in fp16 (integers < 2048); v is fp16-rounded
once (rel ~2^-11); PSUM accumulates in f32. Measured max rel err ~3e-4.
"""
import numpy as np

import concourse.bacc as bacc
import concourse.mybir as mybir
import concourse.tile as tile
from concourse.bass2jax import run_bass_via_pjrt

B, H, W, C = 32, 112, 112, 64
NPOS = H * W                 # 12544 positions per batch
NCHUNK = NPOS // 128         # 98 chunks per plane
LO, HI = 128, 392            # 50176 = LO * HI bin decomposition
OUT_HW = (2 * H) * (2 * W)   # 50176
N_CORES = 8
AL = mybir.AluOpType

_cached_nc = None


def _build_module():
    """One invocation: 1 batch (64 planes) on one core."""
    nc = bacc.Bacc("TRN2", target_bir_lowering=False, debug=False)
    v_d = nc.dram_tensor("v", [128, C * NCHUNK], mybir.dt.float32,
                         kind="ExternalInput")
    m_d = nc.dram_tensor("m", [128, C * NCHUNK], mybir.dt.int32,
                         kind="ExternalInput")
    io392_d = nc.dram_tensor("io392", [128, HI], mybir.dt.float16,
                             kind="ExternalInput")
    io128_d = nc.dram_tensor("io128", [128, LO], mybir.dt.float16,
                             kind="ExternalInput")
    out_d = nc.dram_tensor("out", [128, C * HI], mybir.dt.float32,
                           kind="ExternalOutput")

    with tile.TileContext(nc) as tc:
        with tc.tile_pool(name="sbuf", bufs=1) as pool, \
             tc.tile_pool(name="dec", bufs=3) as decpool, \
             tc.tile_pool(name="pp", bufs=4, space="PSUM") as ppool, \
             tc.tile_pool(name="st", bufs=3) as stpool, \
             tc.tile_pool(name="ab", bufs=6) as abpool:
            v = pool.tile([128, C * NCHUNK], mybir.dt.float32)
            mi = pool.tile([128, C * NCHUNK], mybir.dt.int32)
            io392 = pool.tile([128, HI], mybir.dt.float16)
            io128 = pool.tile([128, LO], mybir.dt.float16)
            for t_, d_ in [(v, v_d), (mi, m_d), (io392, io392_d),
                           (io128, io128_d)]:
                nc.sync.dma_start(out=t_[:], in_=d_[:])

            for p in range(C):
                sl = slice(p * NCHUNK, (p + 1) * NCHUNK)
                # decode bin = mask>>6 -> lo = bin//392 (f32 reciprocal with
                # half-bin bias; f32->int convert is round-to-nearest, so the
                # -0.5 makes it a floor), hi = bin - 392*lo. All values are
                # integers exactly representable in f32.
                t_i = decpool.tile([128, NCHUNK], mybir.dt.int32, name="ti")
                t_f = decpool.tile([128, NCHUNK], mybir.dt.float32, name="tf")
                qf = decpool.tile([128, NCHUNK], mybir.dt.float32, name="qf")
                qi = decpool.tile([128, NCHUNK], mybir.dt.int32, name="qi")
                lo_f = decpool.tile([128, NCHUNK], mybir.dt.float32, name="lo")
                hi_f = decpool.tile([128, NCHUNK], mybir.dt.float32, name="hi")
                g = nc.vector
                g.tensor_scalar(out=t_i[:], in0=mi[:, sl], scalar1=6,
                                scalar2=None, op0=AL.logical_shift_right)
                g.tensor_scalar(out=t_f[:], in0=t_i[:], scalar1=0,
                                scalar2=None, op0=AL.add)
                g.tensor_scalar(out=qf[:], in0=t_f[:], scalar1=float(1.0 / 392),
                                scalar2=float(0.5 / 392 - 0.5),
                                op0=AL.mult, op1=AL.add)
                g.tensor_scalar(out=qi[:], in0=qf[:], scalar1=0, scalar2=None,
                                op0=AL.add)
                g.tensor_scalar(out=lo_f[:], in0=qi[:], scalar1=0, scalar2=None,
                                op0=AL.add)
                g.scalar_tensor_tensor(out=hi_f[:], in0=lo_f[:], scalar=-392.0,
                                       in1=t_f[:], op0=AL.mult, op1=AL.add)

                psum = ppool.tile([128, HI], mybir.dt.float32, name="ps")
                for k in range(NCHUNK):
                    A = abpool.tile([128, LO], mybir.dt.float16, name="A")
                    M = abpool.tile([128, HI], mybir.dt.float16, name="M")
                    nc.vector.tensor_scalar(out=M[:], in0=io392[:],
                                            scalar1=hi_f[:, k:k + 1],
                                            scalar2=None, op0=AL.is_equal)
                    nc.vector.tensor_scalar(out=A[:], in0=io128[:],
                                        scalar1=lo_f[:, k:k + 1],
                                        scalar2=v[:, sl][:, k:k + 1],
                                        op0=AL.is_equal, op1=AL.mult)
                    nc.tensor.matmul(out=psum[:], lhsT=A[:], rhs=M[:],
                                     start=(k == 0), stop=(k == NCHUNK - 1))
                stage = stpool.tile([128, HI], mybir.dt.float32, name="sg")
                nc.vector.tensor_copy(out=stage[:], in_=psum[:])
                nc.sync.dma_start(out=out_d[:, p * HI:(p + 1) * HI],
                                  in_=stage[:])
    nc.compile()
    return nc


def _get_module():
    global _cached_nc
    if _cached_nc is None:
        _cached_nc = _build_module()
    return _cached_nc


def _iotas():
    io392 = np.broadcast_to(np.arange(HI, dtype=np.float16), (128, HI)).copy()
    io128 = np.broadcast_to(np.arange(LO, dtype=np.float16), (128, LO)).copy()
    return io392, io128


def kernel(updates: np.ndarray, mask: np.ndarray) -> np.ndarray:
    assert updates.shape == (B, H, W, C) and mask.shape == (B, H, W, C)
    updates = np.ascontiguousarray(updates, dtype=np.float32)
    mask = np.ascontiguousarray(mask, dtype=np.int32)

    # host layout (data-independent): [B, NPOS, C] -> per batch
    # [128 lane, C * NCHUNK] with column (c*NCHUNK + k) = chunk k of plane c,
    # lane i = position k*128+i.
    upd_t = updates.reshape(B, NCHUNK, 128, C).transpose(0, 2, 3, 1)
    msk_t = mask.reshape(B, NCHUNK, 128, C).transpose(0, 2, 3, 1)
    upd_t = np.ascontiguousarray(upd_t).reshape(B, 128, C * NCHUNK)
    msk_t = np.ascontiguousarray(msk_t).reshape(B, 128, C * NCHUNK)

    io392, io128 = _iotas()
    nc = _get_module()

    # 32 batches over 8 cores x 4 rounds
    outs = np.empty((B, 128, C * HI), dtype=np.float32)
    for rnd in range(4):
        in_maps = []
        for core in range(N_CORES):
            b = rnd * N_CORES + core
            in_maps.append({
                "v": upd_t[b],
                "m": msk_t[b],
                "io392": io392,
                "io128": io128,
            })
        results = run_bass_via_pjrt(nc, in_maps, n_cores=N_CORES)
        for core in range(N_CORES):
            outs[rnd * N_CORES + core] = results[core]["out"]

    # out_d[l, c*HI + h] = plane c bin l*HI+h -> out[b, pix, c]
    # reshape [128 lo, C, HI] -> transpose to [lo, HI, C] -> [50176, C]
    out = outs.reshape(B, 128, C, HI).transpose(0, 1, 3, 2).reshape(
        B, OUT_HW, C)
    return np.ascontiguousarray(out).reshape(B, 2 * H, 2 * W, C)


# revision 6
# speedup vs baseline: 1.0199x; 1.0199x over previous
"""MaxUnpooling2D scatter-add for Trainium2 (8 NeuronCores) — one-hot matmul.

Problem: updates/mask [32,112,112,64] f32/int32 -> out [32,224,224,64] f32,
out[b, y, x, c] += updates[b, h, w, c]; y,x decoded from mask. Per (b,c)
"plane": 12544 elements scatter-add into 50176 pixel bins (bin = mask>>6,
channel index = lane c).

Algorithm (NO per-element DMA): decompose bin t = lo*392 + hi (lo<128,
hi<392). For each 128-element chunk i of a plane build two one-hot fp16
matrices on the DVE (vector) engine:
    A[i, l] = (lo_i == l) * v_i        [128 x 128]   (stationary)
    M[i, h] = (hi_i == h)              [128 x 392]   (moving)
then PE matmul-accumulates PSUM[l, h] += A^T @ M over the plane's 98 chunks:
every element lands exactly at (lo_i, hi_i) with value v_i, duplicates are
summed by the contraction/PSUM accumulate (race-free by construction). The
dense plane is then copied PSUM->SBUF and written with a plain DMA.

Sharding: batch b across 8 cores x 4 sequential invocations of ONE compiled
module (one batch = 64 planes per invocation). Output is written plane-major
[c, lo, hi]; host reassembles to [b, 224*224, 64] (fixed transposes only).

Precision: one-hots are exact in fp16 (integers < 2048); v is fp16-rounded
once (rel ~2^-11); PSUM accumulates in f32. Measured max rel err ~3e-4.
"""
import numpy as np

import concourse.bacc as bacc
import concourse.mybir as mybir
import concourse.tile as tile
from concourse.bass2jax import run_bass_via_pjrt

B, H, W, C = 32, 112, 112, 64
NPOS = H * W                 # 12544 positions per batch
NCHUNK = NPOS // 128         # 98 chunks per plane
LO, HI = 128, 392            # 50176 = LO * HI bin decomposition
OUT_HW = (2 * H) * (2 * W)   # 50176
N_CORES = 8
AL = mybir.AluOpType

_cached_nc = None


def _build_module():
    """One invocation: 1 batch (64 planes) on one core."""
    nc = bacc.Bacc("TRN2", target_bir_lowering=False, debug=False)
    v_d = nc.dram_tensor("v", [128, C * NCHUNK], mybir.dt.float32,
                         kind="ExternalInput")
    m_d = nc.dram_tensor("m", [128, C * NCHUNK], mybir.dt.int32,
                         kind="ExternalInput")
    io392_d = nc.dram_tensor("io392", [128, HI], mybir.dt.float16,
                             kind="ExternalInput")
    io128_d = nc.dram_tensor("io128", [128, LO], mybir.dt.float16,
                             kind="ExternalInput")
    out_d = nc.dram_tensor("out", [128, C * HI], mybir.dt.float32,
                           kind="ExternalOutput")

    with tile.TileContext(nc) as tc:
        with tc.tile_pool(name="sbuf", bufs=1) as pool, \
             tc.tile_pool(name="dec", bufs=3) as decpool, \
             tc.tile_pool(name="pp", bufs=4, space="PSUM") as ppool, \
             tc.tile_pool(name="st", bufs=3) as stpool, \
             tc.tile_pool(name="ab", bufs=6) as abpool:
            v = pool.tile([128, C * NCHUNK], mybir.dt.float32)
            mi = pool.tile([128, C * NCHUNK], mybir.dt.int32)
            io392 = pool.tile([128, HI], mybir.dt.float16)
            io128 = pool.tile([128, LO], mybir.dt.float16)
            for t_, d_ in [(v, v_d), (mi, m_d), (io392, io392_d),
                           (io128, io128_d)]:
                nc.sync.dma_start(out=t_[:], in_=d_[:])

            for p in range(C):
                sl = slice(p * NCHUNK, (p + 1) * NCHUNK)
                # decode bin = mask>>6 -> lo = bin//392 (f32 reciprocal with
                # half-bin bias; f32->int convert is round-to-nearest, so the
                # -0.5 makes it a floor), hi = bin - 392*lo. All values are
                # integers exactly representable in f32.
                t_i = decpool.tile([128, NCHUNK], mybir.dt.int32, name="ti")
                t_f = decpool.tile([128, NCHUNK], mybir.dt.float32, name="tf")
                qf = decpool.tile([128, NCHUNK], mybir.dt.float32, name="qf")
                qi = decpool.tile([128, NCHUNK], mybir.dt.int32, name="qi")
                lo_f = decpool.tile([128, NCHUNK], mybir.dt.float32, name="lo")
                hi_f = decpool.tile([128, NCHUNK], mybir.dt.float32, name="hi")
                g = nc.vector
                g.tensor_scalar(out=t_i[:], in0=mi[:, sl], scalar1=6,
                                scalar2=None, op0=AL.logical_shift_right)
                g.tensor_scalar(out=t_f[:], in0=t_i[:], scalar1=0,
                                scalar2=None, op0=AL.add)
                g.tensor_scalar(out=qf[:], in0=t_f[:], scalar1=float(1.0 / 392),
                                scalar2=float(0.5 / 392 - 0.5),
                                op0=AL.mult, op1=AL.add)
                g.tensor_scalar(out=qi[:], in0=qf[:], scalar1=0, scalar2=None,
                                op0=AL.add)
                g.tensor_scalar(out=lo_f[:], in0=qi[:], scalar1=0, scalar2=None,
                                op0=AL.add)
                g.scalar_tensor_tensor(out=hi_f[:], in0=lo_f[:], scalar=-392.0,
                                       in1=t_f[:], op0=AL.mult, op1=AL.add)

                psum = ppool.tile([128, HI], mybir.dt.float32, name="ps")
                for k in range(NCHUNK):
                    A = abpool.tile([128, LO], mybir.dt.float16, name="A")
                    M = abpool.tile([128, HI], mybir.dt.float16, name="M")
                    nc.vector.tensor_scalar(out=M[:], in0=io392[:],
                                            scalar1=hi_f[:, k:k + 1],
                                            scalar2=None, op0=AL.is_equal)
                    nc.vector.tensor_scalar(out=A[:], in0=io128[:],
                                        scalar1=lo_f[:, k:k + 1],
                                        scalar2=v[:, sl][:, k:k + 1],
                                        op0=AL.is_equal, op1=AL.mult)
                    nc.tensor.matmul(out=psum[:], lhsT=A[:], rhs=M[:],
                                     start=(k == 0), stop=(k == NCHUNK - 1))
                stage = stpool.tile([128, HI], mybir.dt.float32, name="sg")
                # PSUM->SBUF evacuation on ACT frees the DVE (the bottleneck
                # engine) of ~5ns/chunk; plain f32 Copy, no numeric change
                nc.scalar.copy(stage[:], psum[:])
                nc.sync.dma_start(out=out_d[:, p * HI:(p + 1) * HI],
                                  in_=stage[:])
    nc.compile()
    return nc


def _get_module():
    global _cached_nc
    if _cached_nc is None:
        _cached_nc = _build_module()
    return _cached_nc


def _iotas():
    io392 = np.broadcast_to(np.arange(HI, dtype=np.float16), (128, HI)).copy()
    io128 = np.broadcast_to(np.arange(LO, dtype=np.float16), (128, LO)).copy()
    return io392, io128


def kernel(updates: np.ndarray, mask: np.ndarray) -> np.ndarray:
    assert updates.shape == (B, H, W, C) and mask.shape == (B, H, W, C)
    updates = np.ascontiguousarray(updates, dtype=np.float32)
    mask = np.ascontiguousarray(mask, dtype=np.int32)

    # host layout (data-independent): [B, NPOS, C] -> per batch
    # [128 lane, C * NCHUNK] with column (c*NCHUNK + k) = chunk k of plane c,
    # lane i = position k*128+i.
    upd_t = updates.reshape(B, NCHUNK, 128, C).transpose(0, 2, 3, 1)
    msk_t = mask.reshape(B, NCHUNK, 128, C).transpose(0, 2, 3, 1)
    upd_t = np.ascontiguousarray(upd_t).reshape(B, 128, C * NCHUNK)
    msk_t = np.ascontiguousarray(msk_t).reshape(B, 128, C * NCHUNK)

    io392, io128 = _iotas()
    nc = _get_module()

    # 32 batches over 8 cores x 4 rounds
    outs = np.empty((B, 128, C * HI), dtype=np.float32)
    for rnd in range(4):
        in_maps = []
        for core in range(N_CORES):
            b = rnd * N_CORES + core
            in_maps.append({
                "v": upd_t[b],
                "m": msk_t[b],
                "io392": io392,
                "io128": io128,
            })
        results = run_bass_via_pjrt(nc, in_maps, n_cores=N_CORES)
        for core in range(N_CORES):
            outs[rnd * N_CORES + core] = results[core]["out"]

    # out_d[l, c*HI + h] = plane c bin l*HI+h -> out[b, pix, c]
    # reshape [128 lo, C, HI] -> transpose to [lo, HI, C] -> [50176, C]
    out = outs.reshape(B, 128, C, HI).transpose(0, 1, 3, 2).reshape(
        B, OUT_HW, C)
    return np.ascontiguousarray(out).reshape(B, 2 * H, 2 * W, C)


# revision 7
# speedup vs baseline: 1.0336x; 1.0134x over previous
"""MaxUnpooling2D scatter-add for Trainium2 (8 NeuronCores) — one-hot matmul.

Problem: updates/mask [32,112,112,64] f32/int32 -> out [32,224,224,64] f32,
out[b, y, x, c] += updates[b, h, w, c]; y,x decoded from mask. Per (b,c)
"plane": 12544 elements scatter-add into 50176 pixel bins (bin = mask>>6,
channel index = lane c).

Algorithm (NO per-element DMA): decompose bin t = lo*392 + hi (lo<128,
hi<392). For each 128-element chunk i of a plane build two one-hot fp16
matrices on the DVE (vector) engine:
    A[i, l] = (lo_i == l) * v_i        [128 x 128]   (stationary)
    M[i, h] = (hi_i == h)              [128 x 392]   (moving)
then PE matmul-accumulates PSUM[l, h] += A^T @ M over the plane's 98 chunks:
every element lands exactly at (lo_i, hi_i) with value v_i, duplicates are
summed by the contraction/PSUM accumulate (race-free by construction). The
dense plane is then copied PSUM->SBUF and written with a plain DMA.

Sharding: batch b across 8 cores x 4 sequential invocations of ONE compiled
module (one batch = 64 planes per invocation). Output is written plane-major
[c, lo, hi]; host reassembles to [b, 224*224, 64] (fixed transposes only).

Precision: one-hots are exact in fp16 (integers < 2048); v is fp16-rounded
once (rel ~2^-11); PSUM accumulates in f32. Measured max rel err ~3e-4.
"""
import numpy as np

import concourse.bacc as bacc
import concourse.mybir as mybir
import concourse.tile as tile
from concourse.bass2jax import run_bass_via_pjrt

B, H, W, C = 32, 112, 112, 64
NPOS = H * W                 # 12544 positions per batch
NCHUNK = NPOS // 128         # 98 chunks per plane
LO, HI = 128, 392            # 50176 = LO * HI bin decomposition
OUT_HW = (2 * H) * (2 * W)   # 50176
N_CORES = 8
AL = mybir.AluOpType

_cached_nc = None


def _build_module():
    """One invocation: 1 batch (64 planes) on one core."""
    nc = bacc.Bacc("TRN2", target_bir_lowering=False, debug=False)
    v_d = nc.dram_tensor("v", [128, C * NCHUNK], mybir.dt.float32,
                         kind="ExternalInput")
    m_d = nc.dram_tensor("m", [128, C * NCHUNK], mybir.dt.int32,
                         kind="ExternalInput")
    io392_d = nc.dram_tensor("io392", [128, HI], mybir.dt.float16,
                             kind="ExternalInput")
    io128_d = nc.dram_tensor("io128", [128, LO], mybir.dt.float16,
                             kind="ExternalInput")
    out_d = nc.dram_tensor("out", [128, C * HI], mybir.dt.float32,
                           kind="ExternalOutput")

    with tile.TileContext(nc) as tc:
        with tc.tile_pool(name="sbuf", bufs=1) as pool, \
             tc.tile_pool(name="dec", bufs=2) as decpool, \
             tc.tile_pool(name="pp", bufs=4, space="PSUM") as ppool, \
             tc.tile_pool(name="st", bufs=3) as stpool, \
             tc.tile_pool(name="ab", bufs=6) as abpool:
            v = pool.tile([128, C * NCHUNK], mybir.dt.float32)
            mi = pool.tile([128, C * NCHUNK], mybir.dt.int32)
            io392 = pool.tile([128, HI], mybir.dt.float16)
            io128 = pool.tile([128, LO], mybir.dt.float16)
            for t_, d_ in [(v, v_d), (mi, m_d), (io392, io392_d),
                           (io128, io128_d)]:
                nc.sync.dma_start(out=t_[:], in_=d_[:])

            # decode hoisted to module level (2 half-passes instead of
            # 6 ops x 64 planes: saves ~380 DVE instruction fixed costs).
            # bin = mask>>6 -> lo = bin//392 (f32 reciprocal with half-bin
            # bias; the f32->int convert is round-to-nearest, so the -0.5
            # makes it a floor), hi = bin - 392*lo. All integers exact in f32.
            lo_f = pool.tile([128, C * NCHUNK], mybir.dt.float32)
            hi_f = pool.tile([128, C * NCHUNK], mybir.dt.float32)
            half = C * NCHUNK // 2
            for h in range(2):
                s = slice(h * half, (h + 1) * half)
                t_i = decpool.tile([128, half], mybir.dt.int32, name="ti")
                t_f = decpool.tile([128, half], mybir.dt.float32, name="tf")
                qf = decpool.tile([128, half], mybir.dt.float32, name="qf")
                qi = decpool.tile([128, half], mybir.dt.int32, name="qi")
                g = nc.vector
                g.tensor_scalar(out=t_i[:], in0=mi[:, s], scalar1=6,
                                scalar2=None, op0=AL.logical_shift_right)
                g.tensor_scalar(out=t_f[:], in0=t_i[:], scalar1=0,
                                scalar2=None, op0=AL.add)
                g.tensor_scalar(out=qf[:], in0=t_f[:], scalar1=float(1.0 / 392),
                                scalar2=float(0.5 / 392 - 0.5),
                                op0=AL.mult, op1=AL.add)
                g.tensor_scalar(out=qi[:], in0=qf[:], scalar1=0, scalar2=None,
                                op0=AL.add)
                g.tensor_scalar(out=lo_f[:, s], in0=qi[:], scalar1=0,
                                scalar2=None, op0=AL.add)
                g.scalar_tensor_tensor(out=hi_f[:, s], in0=lo_f[:, s],
                                       scalar=-392.0, in1=t_f[:],
                                       op0=AL.mult, op1=AL.add)

            for p in range(C):
                psum = ppool.tile([128, HI], mybir.dt.float32, name="ps")
                for k in range(NCHUNK):
                    col = p * NCHUNK + k
                    A = abpool.tile([128, LO], mybir.dt.float16, name="A")
                    M = abpool.tile([128, HI], mybir.dt.float16, name="M")
                    nc.vector.tensor_scalar(out=M[:], in0=io392[:],
                                            scalar1=hi_f[:, col:col + 1],
                                            scalar2=None, op0=AL.is_equal)
                    nc.vector.tensor_scalar(out=A[:], in0=io128[:],
                                            scalar1=lo_f[:, col:col + 1],
                                            scalar2=v[:, col:col + 1],
                                            op0=AL.is_equal, op1=AL.mult)
                    nc.tensor.matmul(out=psum[:], lhsT=A[:], rhs=M[:],
                                     start=(k == 0), stop=(k == NCHUNK - 1))
                stage = stpool.tile([128, HI], mybir.dt.float32, name="sg")
                # PSUM->SBUF evacuation on ACT frees the DVE (the bottleneck
                # engine); plain f32 Copy, no numeric change
                nc.scalar.copy(stage[:], psum[:])
                nc.sync.dma_start(out=out_d[:, p * HI:(p + 1) * HI],
                                  in_=stage[:])
    nc.compile()
    return nc


def _get_module():
    global _cached_nc
    if _cached_nc is None:
        _cached_nc = _build_module()
    return _cached_nc


def _iotas():
    io392 = np.broadcast_to(np.arange(HI, dtype=np.float16), (128, HI)).copy()
    io128 = np.broadcast_to(np.arange(LO, dtype=np.float16), (128, LO)).copy()
    return io392, io128


def kernel(updates: np.ndarray, mask: np.ndarray) -> np.ndarray:
    assert updates.shape == (B, H, W, C) and mask.shape == (B, H, W, C)
    updates = np.ascontiguousarray(updates, dtype=np.float32)
    mask = np.ascontiguousarray(mask, dtype=np.int32)

    # host layout (data-independent): [B, NPOS, C] -> per batch
    # [128 lane, C * NCHUNK] with column (c*NCHUNK + k) = chunk k of plane c,
    # lane i = position k*128+i.
    upd_t = updates.reshape(B, NCHUNK, 128, C).transpose(0, 2, 3, 1)
    msk_t = mask.reshape(B, NCHUNK, 128, C).transpose(0, 2, 3, 1)
    upd_t = np.ascontiguousarray(upd_t).reshape(B, 128, C * NCHUNK)
    msk_t = np.ascontiguousarray(msk_t).reshape(B, 128, C * NCHUNK)

    io392, io128 = _iotas()
    nc = _get_module()

    # 32 batches over 8 cores x 4 rounds
    outs = np.empty((B, 128, C * HI), dtype=np.float32)
    for rnd in range(4):
        in_maps = []
        for core in range(N_CORES):
            b = rnd * N_CORES + core
            in_maps.append({
                "v": upd_t[b],
                "m": msk_t[b],
                "io392": io392,
                "io128": io128,
            })
        results = run_bass_via_pjrt(nc, in_maps, n_cores=N_CORES)
        for core in range(N_CORES):
            outs[rnd * N_CORES + core] = results[core]["out"]

    # out_d[l, c*HI + h] = plane c bin l*HI+h -> out[b, pix, c]
    # reshape [128 lo, C, HI] -> transpose to [lo, HI, C] -> [50176, C]
    out = outs.reshape(B, 128, C, HI).transpose(0, 1, 3, 2).reshape(
        B, OUT_HW, C)
    return np.ascontiguousarray(out).reshape(B, 2 * H, 2 * W, C)


# revision 8
# speedup vs baseline: 1.0372x; 1.0035x over previous
"""MaxUnpooling2D scatter-add for Trainium2 (8 NeuronCores) — one-hot matmul.

Problem: updates/mask [32,112,112,64] f32/int32 -> out [32,224,224,64] f32,
out[b, y, x, c] += updates[b, h, w, c]; y,x decoded from mask. Per (b,c)
"plane": 12544 elements scatter-add into 50176 pixel bins (bin = mask>>6,
channel index = lane c).

Algorithm (NO per-element DMA): decompose bin t = lo*392 + hi (lo<128,
hi<392). For each 128-element chunk i of a plane build two one-hot fp16
matrices on the DVE (vector) engine:
    A[i, l] = (lo_i == l) * v_i        [128 x 128]   (stationary)
    M[i, h] = (hi_i == h)              [128 x 392]   (moving)
then PE matmul-accumulates PSUM[l, h] += A^T @ M over the plane's 98 chunks:
every element lands exactly at (lo_i, hi_i) with value v_i, duplicates are
summed by the contraction/PSUM accumulate (race-free by construction). The
dense plane is then copied PSUM->SBUF and written with a plain DMA.

Sharding: batch b across 8 cores x 4 sequential invocations of ONE compiled
module (one batch = 64 planes per invocation). Output is written plane-major
[c, lo, hi]; host reassembles to [b, 224*224, 64] (fixed transposes only).

Precision: one-hots are exact in fp16 (integers < 2048); v is fp16-rounded
once (rel ~2^-11); PSUM accumulates in f32. Measured max rel err ~3e-4.
"""
import numpy as np

import concourse.bacc as bacc
import concourse.mybir as mybir
import concourse.tile as tile
from concourse.bass2jax import run_bass_via_pjrt

B, H, W, C = 32, 112, 112, 64
NPOS = H * W                 # 12544 positions per batch
NCHUNK = NPOS // 128         # 98 chunks per plane
LO, HI = 128, 392            # 50176 = LO * HI bin decomposition
OUT_HW = (2 * H) * (2 * W)   # 50176
N_CORES = 8
AL = mybir.AluOpType

_cached_nc = None


def _build_module():
    """One invocation: 1 batch (64 planes) on one core."""
    nc = bacc.Bacc("TRN2", target_bir_lowering=False, debug=False)
    v_d = nc.dram_tensor("v", [128, C * NCHUNK], mybir.dt.float32,
                         kind="ExternalInput")
    m_d = nc.dram_tensor("m", [128, C * NCHUNK], mybir.dt.int32,
                         kind="ExternalInput")
    io392_d = nc.dram_tensor("io392", [128, HI], mybir.dt.float16,
                             kind="ExternalInput")
    io128_d = nc.dram_tensor("io128", [128, LO], mybir.dt.float16,
                             kind="ExternalInput")
    out_d = nc.dram_tensor("out", [128, C * HI], mybir.dt.float32,
                           kind="ExternalOutput")

    with tile.TileContext(nc) as tc:
        with tc.tile_pool(name="sbuf", bufs=1) as pool, \
             tc.tile_pool(name="dec", bufs=2) as decpool, \
             tc.tile_pool(name="pp", bufs=4, space="PSUM") as ppool, \
             tc.tile_pool(name="st", bufs=3) as stpool, \
             tc.tile_pool(name="ab", bufs=6) as abpool:
            v = pool.tile([128, C * NCHUNK], mybir.dt.float32)
            mi = pool.tile([128, C * NCHUNK], mybir.dt.int32)
            io392 = pool.tile([128, HI], mybir.dt.float16)
            io128 = pool.tile([128, LO], mybir.dt.float16)
            for t_, d_ in [(v, v_d), (io392, io392_d), (io128, io128_d)]:
                nc.sync.dma_start(out=t_[:], in_=d_[:])

            # decode hoisted to module level (2 half-passes instead of
            # 6 ops x 64 planes: saves ~380 DVE instruction fixed costs).
            # bin = mask>>6 -> lo = bin//392 (f32 reciprocal with half-bin
            # bias; the f32->int convert is round-to-nearest, so the -0.5
            # makes it a floor), hi = bin - 392*lo. All integers exact in f32.
            lo_f = pool.tile([128, C * NCHUNK], mybir.dt.float32)
            hi_f = pool.tile([128, C * NCHUNK], mybir.dt.float32)
            # mask load and decode split into quarters so the first
            # plane's chunks start after ~1/4 of the prologue
            part = C * NCHUNK // 4
            for h in range(4):
                s = slice(h * part, (h + 1) * part)
                nc.sync.dma_start(out=mi[:, s], in_=m_d[:, s])
                t_i = decpool.tile([128, part], mybir.dt.int32, name="ti")
                t_f = decpool.tile([128, part], mybir.dt.float32, name="tf")
                qf = decpool.tile([128, part], mybir.dt.float32, name="qf")
                qi = decpool.tile([128, part], mybir.dt.int32, name="qi")
                g = nc.vector
                g.tensor_scalar(out=t_i[:], in0=mi[:, s], scalar1=6,
                                scalar2=None, op0=AL.logical_shift_right)
                g.tensor_scalar(out=t_f[:], in0=t_i[:], scalar1=0,
                                scalar2=None, op0=AL.add)
                g.tensor_scalar(out=qf[:], in0=t_f[:], scalar1=float(1.0 / 392),
                                scalar2=float(0.5 / 392 - 0.5),
                                op0=AL.mult, op1=AL.add)
                g.tensor_scalar(out=qi[:], in0=qf[:], scalar1=0, scalar2=None,
                                op0=AL.add)
                g.tensor_scalar(out=lo_f[:, s], in0=qi[:], scalar1=0,
                                scalar2=None, op0=AL.add)
                g.scalar_tensor_tensor(out=hi_f[:, s], in0=lo_f[:, s],
                                       scalar=-392.0, in1=t_f[:],
                                       op0=AL.mult, op1=AL.add)

            for p in range(C):
                psum = ppool.tile([128, HI], mybir.dt.float32, name="ps")
                for k in range(NCHUNK):
                    col = p * NCHUNK + k
                    A = abpool.tile([128, LO], mybir.dt.float16, name="A")
                    M = abpool.tile([128, HI], mybir.dt.float16, name="M")
                    nc.vector.tensor_scalar(out=M[:], in0=io392[:],
                                            scalar1=hi_f[:, col:col + 1],
                                            scalar2=None, op0=AL.is_equal)
                    nc.vector.tensor_scalar(out=A[:], in0=io128[:],
                                            scalar1=lo_f[:, col:col + 1],
                                            scalar2=v[:, col:col + 1],
                                            op0=AL.is_equal, op1=AL.mult)
                    nc.tensor.matmul(out=psum[:], lhsT=A[:], rhs=M[:],
                                     start=(k == 0), stop=(k == NCHUNK - 1))
                stage = stpool.tile([128, HI], mybir.dt.float32, name="sg")
                # PSUM->SBUF evacuation on ACT frees the DVE (the bottleneck
                # engine); plain f32 Copy, no numeric change
                nc.scalar.copy(stage[:], psum[:])
                nc.sync.dma_start(out=out_d[:, p * HI:(p + 1) * HI],
                                  in_=stage[:])
    nc.compile()
    return nc


def _get_module():
    global _cached_nc
    if _cached_nc is None:
        _cached_nc = _build_module()
    return _cached_nc


def _iotas():
    io392 = np.broadcast_to(np.arange(HI, dtype=np.float16), (128, HI)).copy()
    io128 = np.broadcast_to(np.arange(LO, dtype=np.float16), (128, LO)).copy()
    return io392, io128


def kernel(updates: np.ndarray, mask: np.ndarray) -> np.ndarray:
    assert updates.shape == (B, H, W, C) and mask.shape == (B, H, W, C)
    updates = np.ascontiguousarray(updates, dtype=np.float32)
    mask = np.ascontiguousarray(mask, dtype=np.int32)

    # host layout (data-independent): [B, NPOS, C] -> per batch
    # [128 lane, C * NCHUNK] with column (c*NCHUNK + k) = chunk k of plane c,
    # lane i = position k*128+i.
    upd_t = updates.reshape(B, NCHUNK, 128, C).transpose(0, 2, 3, 1)
    msk_t = mask.reshape(B, NCHUNK, 128, C).transpose(0, 2, 3, 1)
    upd_t = np.ascontiguousarray(upd_t).reshape(B, 128, C * NCHUNK)
    msk_t = np.ascontiguousarray(msk_t).reshape(B, 128, C * NCHUNK)

    io392, io128 = _iotas()
    nc = _get_module()

    # 32 batches over 8 cores x 4 rounds
    outs = np.empty((B, 128, C * HI), dtype=np.float32)
    for rnd in range(4):
        in_maps = []
        for core in range(N_CORES):
            b = rnd * N_CORES + core
            in_maps.append({
                "v": upd_t[b],
                "m": msk_t[b],
                "io392": io392,
                "io128": io128,
            })
        results = run_bass_via_pjrt(nc, in_maps, n_cores=N_CORES)
        for core in range(N_CORES):
            outs[rnd * N_CORES + core] = results[core]["out"]

    # out_d[l, c*HI + h] = plane c bin l*HI+h -> out[b, pix, c]
    # reshape [128 lo, C, HI] -> transpose to [lo, HI, C] -> [50176, C]
    out = outs.reshape(B, 128, C, HI).transpose(0, 1, 3, 2).reshape(
        B, OUT_HW, C)
    return np.ascontiguousarray(out).reshape(B, 2 * H, 2 * W, C)
